# revision 11
# baseline (speedup 1.0000x reference)
"""CosRec-style pairwise-MLP recommender kernel for 8 Trainium2 NeuronCores.

Reference computation (per batch element b, L=32, D=64, FC=100):
    embs   = item_emb[seq_var]                      [B, L, D]
    A      = embs @ Wa^T  (Wa = W1[:, :D])          [B, L, FC]
    Bm     = embs @ Wb^T  (Wb = W1[:, D:])          [B, L, FC]
    h1     = relu(A[:,None,:,:] + Bm[:,:,None,:] + b1)   [B, L, L, FC]
    h2     = relu(h1 @ Wf2^T + bf2)                 [B, L, L, FC]
    x      = h2.sum((1, 2))                         [B, FC]
    out[b,t] = b2[item_var[b,t]] + W2[item_var[b,t]] . cat(x[b], user_emb[user_var[b]])

Strategy: data-parallel over batch (64 examples/core).  All gathers are done
on-device with indirect DMA; the [64, 32, 32, 100] per-core h tensor never
touches HBM — it lives tile-by-tile in SBUF/PSUM.  Per example:
  DVE   : pre = A'[:,c] + Bm[:,a]  (outer sum via broadcast APs, [100, 1024])
  DVE/ACT: relu in-place
  PE    : h2 = Wf2T.T @ h1 into PSUM (2 x N=512 matmuls, fp32r by default)
  ACT   : relu(h2 + bf2) with fused accumulate -> x[:, b]  (one instruction)
Final stage: per-target-t fused multiply-reduce on DVE against gathered W2
rows, with b2 as the reduction seed.
"""

import os
import sys

import numpy as np

sys.path.insert(0, "/opt/trn_rl_repo")

import concourse.bass as bass
import concourse.tile as tile
from concourse import bacc, mybir
from concourse.bass_utils import run_bass_kernel_spmd
from concourse.masks import make_identity
from contextlib import ExitStack

N_CORES = 8
B_FULL = 512
BPC = B_FULL // N_CORES  # 64 examples per core
L = 32
D = 64
FC = 100
T = 3
NROW = BPC * L           # 2048 gathered rows per core
NTILE = NROW // 128      # 16 gather tiles
F32 = mybir.dt.float32
I32 = mybir.dt.int32

# ---- tunables -------------------------------------------------------------
CFG = dict(
    r1_act_num=7,      # of every 16 examples, how many run relu1 on ScalarE (rest DVE)
    p3_dve_num=0,      # of every 16 examples, how many run relu2+accum on DVE (rest ACT)
    l2_f32r=True,      # layer-2 matmul in fp32r (4x faster PE, TF32-like precision)
    l1_f32r=False,     # layer-1 matmul in fp32r
    h2_bufs=2,         # PSUM double-buffering for the [100, 1024] h2 tile
    pre_bufs=3,
)

_PROG_CACHE = {}


def _build_program(cfg):
    nc = bacc.Bacc()

    seq_idx = nc.dram_tensor("seq_idx", [128, NTILE], I32, kind="ExternalInput")
    user_idx = nc.dram_tensor("user_idx", [BPC, 1], I32, kind="ExternalInput")
    item_idx = nc.dram_tensor("item_idx", [BPC, T], I32, kind="ExternalInput")
    item_emb = nc.dram_tensor("item_emb", [100000, D], F32, kind="ExternalInput")
    user_emb = nc.dram_tensor("user_emb", [100000, D], F32, kind="ExternalInput")
    W2 = nc.dram_tensor("W2", [100000, FC + D], F32, kind="ExternalInput")
    b2 = nc.dram_tensor("b2", [100000, 1], F32, kind="ExternalInput")
    W1 = nc.dram_tensor("W1", [FC, 2 * D], F32, kind="ExternalInput")
    b1 = nc.dram_tensor("b1", [FC, 1], F32, kind="ExternalInput")
    Wf2 = nc.dram_tensor("Wf2", [FC, FC], F32, kind="ExternalInput")
    bf2 = nc.dram_tensor("bf2", [FC, 1], F32, kind="ExternalInput")
    out_d = nc.dram_tensor("out", [BPC, T], F32, kind="ExternalOutput")

    Relu = mybir.ActivationFunctionType.Relu
    Add = mybir.AluOpType.add
    Mult = mybir.AluOpType.mult
    Max = mybir.AluOpType.max

    r1_act = [(i % 16) < cfg["r1_act_num"] for i in range(BPC)]
    p3_dve = [(i % 16) < cfg["p3_dve_num"] for i in range(BPC)]

    with ExitStack() as ctx:
        tc = ctx.enter_context(tile.TileContext(nc))
        const = ctx.enter_context(tc.tile_pool(name="const", bufs=1))
        gat = ctx.enter_context(tc.tile_pool(name="gat", bufs=4))
        prep = ctx.enter_context(tc.tile_pool(name="pre", bufs=cfg["pre_bufs"]))
        scrp = ctx.enter_context(tc.tile_pool(name="scr", bufs=2))
        ps_m = ctx.enter_context(tc.tile_pool(name="psm", bufs=2, space="PSUM"))
        ps_h = ctx.enter_context(
            tc.tile_pool(name="psh", bufs=cfg["h2_bufs"], space="PSUM")
        )

        # ---------------- constants & weights ----------------
        ident = const.tile([128, 128], F32)
        make_identity(nc, ident[:])

        w1_sb = const.tile([FC, 2 * D], F32)
        nc.sync.dma_start(out=w1_sb[:], in_=W1[:, :])
        wf2_sb = const.tile([FC, FC], F32)
        nc.sync.dma_start(out=wf2_sb[:], in_=Wf2[:, :])
        b1_sb = const.tile([FC, 1], F32)
        nc.sync.dma_start(out=b1_sb[:], in_=b1[:, :])
        bf2_sb = const.tile([FC, 1], F32)
        nc.sync.dma_start(out=bf2_sb[:], in_=bf2[:, :])
        idx_sb = const.tile([128, NTILE], I32)
        nc.sync.dma_start(out=idx_sb[:], in_=seq_idx[:, :])
        uidx_sb = const.tile([BPC, 1], I32)
        nc.sync.dma_start(out=uidx_sb[:], in_=user_idx[:, :])
        iidx_sb = const.tile([BPC, T], I32)
        nc.sync.dma_start(out=iidx_sb[:], in_=item_idx[:, :])

        # WaT/WbT: [64, 100] = (W1[:, :D]).T and (W1[:, D:]).T, both at base partition 0
        waT = const.tile([D, FC], F32)
        wbT = const.tile([D, FC], F32)
        for half, dst in ((0, waT), (1, wbT)):
            w1h_ps = ps_m.tile([D, FC], F32, tag="m")
            nc.tensor.transpose(
                w1h_ps[:], w1_sb[:, half * D : (half + 1) * D], ident[:FC, :FC]
            )
            nc.vector.tensor_copy(dst[:], w1h_ps[:])

        # Wf2T: [100, 100] = Wf2.T  (fp32r-rounded when layer-2 runs in fp32r)
        l2dt = mybir.dt.float32r if cfg["l2_f32r"] else F32
        wf2t_ps = ps_m.tile([FC, FC], F32, tag="m")
        nc.tensor.transpose(wf2t_ps[:], wf2_sb[:], ident[:FC, :FC])
        wf2t = const.tile([FC, FC], l2dt)
        nc.vector.tensor_copy(wf2t[:], wf2t_ps[:])

        # ---------------- embedding gather + transpose ----------------
        # embsT[d, b*L + l] = item_emb[seq[b, l], d]
        embsT = const.tile([D, NROW], F32)
        for t in range(NTILE):
            g = gat.tile([128, D], F32)
            nc.gpsimd.indirect_dma_start(
                out=g[:],
                out_offset=None,
                in_=item_emb[:, :],
                in_offset=bass.IndirectOffsetOnAxis(ap=idx_sb[:, t : t + 1], axis=0),
            )
            tp = ps_m.tile([D, 128], F32, tag="m")
            nc.tensor.transpose(tp[:], g[:], ident[:, :])
            nc.vector.tensor_copy(embsT[:, t * 128 : (t + 1) * 128], tp[:])

        # ---------------- gathers for the final stage ----------------
        ug = const.tile([BPC, D], F32)
        nc.gpsimd.indirect_dma_start(
            out=ug[:],
            out_offset=None,
            in_=user_emb[:, :],
            in_offset=bass.IndirectOffsetOnAxis(ap=uidx_sb[:, 0:1], axis=0),
        )
        w2g = []
        for t in range(T):
            w2g_t = const.tile([BPC, FC + D], F32, tag=f"w2g{t}")
            nc.gpsimd.indirect_dma_start(
                out=w2g_t[:],
                out_offset=None,
                in_=W2[:, :],
                in_offset=bass.IndirectOffsetOnAxis(ap=iidx_sb[:, t : t + 1], axis=0),
            )
            w2g.append(w2g_t)
        b2g = const.tile([BPC, T], F32)
        for t in range(T):
            nc.gpsimd.indirect_dma_start(
                out=b2g[:, t : t + 1],
                out_offset=None,
                in_=b2[:, :],
                in_offset=bass.IndirectOffsetOnAxis(ap=iidx_sb[:, t : t + 1], axis=0),
            )

        # ---------------- layer-1: A' = embs@Wa^T + b1, Bm = embs@Wb^T ------
        # A_t/Bm_t: [100(f), 2048(b*L+l)]
        A_t = const.tile([FC, NROW], F32)
        Bm_t = const.tile([FC, NROW], F32)
        l1dt = mybir.dt.float32r if cfg["l1_f32r"] else F32
        for j in range(NROW // 512):
            sl = slice(j * 512, (j + 1) * 512)
            pa = ps_m.tile([FC, 512], F32, tag="m")
            nc.tensor.matmul(
                pa[:],
                lhsT=waT[:].bitcast(l1dt),
                rhs=embsT[:, sl].bitcast(l1dt),
                start=True,
                stop=True,
            )
            # PSUM->SBUF copy with fused +b1 (per-partition bias)
            nc.scalar.activation(
                A_t[:, sl], pa[:], mybir.ActivationFunctionType.Identity,
                bias=b1_sb[:, 0:1],
            )
            pb = ps_m.tile([FC, 512], F32, tag="m")
            nc.tensor.matmul(
                pb[:],
                lhsT=wbT[:].bitcast(l1dt),
                rhs=embsT[:, sl].bitcast(l1dt),
                start=True,
                stop=True,
            )
            nc.scalar.copy(Bm_t[:, sl], pb[:])

        # ---------------- main loop over examples ----------------
        x = const.tile([FC, BPC], F32)  # x[:, b] = sum_{a,c} h2[b, a, c, :]
        for b in range(BPC):
            sl = slice(b * L, (b + 1) * L)
            pre = prep.tile([FC, L * L], l2dt)
            # pre[f, a*L + c] = A'[f, b*L + c] + Bm[f, b*L + a]
            in0 = A_t[:, sl].unsqueeze(1).to_broadcast([FC, L, L])
            in1 = Bm_t[:, sl].unsqueeze(2).to_broadcast([FC, L, L])
            nc.vector.tensor_tensor(
                out=pre[:].rearrange("p (a c) -> p a c", a=L),
                in0=in0,
                in1=in1,
                op=Add,
            )
            # relu in place
            if r1_act[b]:
                nc.scalar.activation(pre[:], pre[:], Relu)
            else:
                nc.vector.tensor_scalar_max(pre[:], pre[:], 0.0)
            # layer 2: h2pre = Wf2T.T @ h1  (PSUM, 2 banks)
            h2p = ps_h.tile([FC, L * L], F32, tag="h2")
            for half in range(2):
                hs = slice(half * 512, (half + 1) * 512)
                nc.tensor.matmul(
                    h2p[:, hs],
                    lhsT=wf2t[:],
                    rhs=pre[:, hs],
                    start=True,
                    stop=True,
                )
            # relu(h2pre + bf2), accumulate over the 1024 pairs -> x[:, b]
            h2s = scrp.tile([FC, L * L], F32, tag="h2s")
            if p3_dve[b]:
                nc.vector.tensor_scalar(
                    out=h2s[:],
                    in0=h2p[:],
                    scalar1=bf2_sb[:, 0:1],
                    scalar2=0.0,
                    op0=Add,
                    op1=Max,
                    accum_out=x[:, b : b + 1],
                )
            else:
                nc.scalar.activation(
                    h2s[:], h2p[:], Relu,
                    bias=bf2_sb[:, 0:1],
                    accum_out=x[:, b : b + 1],
                )

        # ---------------- final: out[b, t] = b2 + W2row . cat(x, uemb) ------
        xT_ps = ps_m.tile([BPC, FC], F32, tag="m")
        nc.tensor.transpose(xT_ps[:], x[:], ident[:FC, :FC])
        xT = const.tile([BPC, FC], F32)
        nc.vector.tensor_copy(xT[:], xT_ps[:])

        # (tensor_tensor_reduce is broken on this runtime — use mult + accum)
        out_sb = const.tile([BPC, T], F32)
        for t in range(T):
            scr = scrp.tile([BPC, FC + D], F32, tag="fin")
            nc.vector.tensor_tensor(
                out=scr[:, 0:FC], in0=w2g[t][:, 0:FC], in1=xT[:], op=Mult
            )
            nc.vector.tensor_tensor(
                out=scr[:, FC:], in0=w2g[t][:, FC:], in1=ug[:], op=Mult
            )
            acc = scrp.tile([BPC, 1], F32, tag="facc")
            dummy = scrp.tile([BPC, FC + D], F32, tag="fdum")
            nc.scalar.activation(
                dummy[:], scr[:], mybir.ActivationFunctionType.Identity,
                accum_out=acc[:],
            )
            nc.vector.tensor_tensor(
                out=out_sb[:, t : t + 1], in0=acc[:], in1=b2g[:, t : t + 1], op=Add
            )
        nc.sync.dma_start(out=out_d[:, :], in_=out_sb[:])

    nc.finalize()
    return nc


def get_program(cfg=None):
    cfg = dict(CFG if cfg is None else cfg)
    key = tuple(sorted(cfg.items()))
    if key not in _PROG_CACHE:
        _PROG_CACHE[key] = _build_program(cfg)
    return _PROG_CACHE[key]


def make_in_maps(inputs):
    """Shard the full-problem inputs into 8 per-core input maps."""
    seq = np.asarray(inputs["seq_var"]).astype(np.int32)
    usr = np.asarray(inputs["user_var"]).astype(np.int32).reshape(B_FULL, 1)
    itm = np.asarray(inputs["item_var"]).astype(np.int32).reshape(B_FULL, T)
    shared = dict(
        item_emb=np.ascontiguousarray(np.asarray(inputs["item_emb"], np.float32)),
        user_emb=np.ascontiguousarray(np.asarray(inputs["user_emb"], np.float32)),
        W2=np.ascontiguousarray(np.asarray(inputs["W2"], np.float32)),
        b2=np.ascontiguousarray(np.asarray(inputs["b2"], np.float32).reshape(-1, 1)),
        W1=np.ascontiguousarray(np.asarray(inputs["W1"], np.float32)),
        b1=np.ascontiguousarray(np.asarray(inputs["b1"], np.float32).reshape(FC, 1)),
        Wf2=np.ascontiguousarray(np.asarray(inputs["Wf2"], np.float32)),
        bf2=np.ascontiguousarray(np.asarray(inputs["bf2"], np.float32).reshape(FC, 1)),
    )
    in_maps = []
    for c in range(N_CORES):
        rows = slice(c * BPC, (c + 1) * BPC)
        flat = seq[rows].reshape(NROW)               # (b*L + l) order
        seq_pm = np.ascontiguousarray(flat.reshape(NTILE, 128).T)  # [128, 16]
        in_maps.append(
            dict(
                shared,
                seq_idx=seq_pm,
                user_idx=np.ascontiguousarray(usr[rows]),
                item_idx=np.ascontiguousarray(itm[rows]),
            )
        )
    return in_maps


def run_sharded(inputs, cfg=None, trace=False, **kwargs):
    nc = get_program(cfg)
    in_maps = make_in_maps(inputs)
    res = run_bass_kernel_spmd(nc, in_maps, list(range(N_CORES)), trace=trace, **kwargs)
    out = np.concatenate([r["out"] for r in res.results], axis=0)
    return out, res


def kernel(**inputs) -> np.ndarray:
    out, _ = run_sharded(inputs)
    return out
